# revision 22
# baseline (speedup 1.0000x reference)
"""Trainium2 Bass kernel for nn_CustomS4.

Pipeline computed by the reference:
    z   = x @ W^T + b                      adapter Linear      [B,T,D]
    xh  = LN(z) * gamma + beta             LayerNorm over D
    u   = xh @ Bm                          input projection    [B,T,N]
    h_T = sum_t u_t A^{T-1-t}              linear scan, final state only
    out = normalize_rows(h_T @ C)          [B, D]

Key reformulations (all verified against the reference to ~1e-6 rel):

1. Only the FINAL scan state is needed and ||A^k|| decays like rho^k with
   rho = spectral_radius(A) ~ 0.5 (A = 0.5/sqrt(N) * randn), so the scan
   truncates to the last T_EFF timesteps with error below fp32 noise.
   T_EFF is chosen on the host from the actual decay of ||A^k||.

2. LayerNorm folds into the weights: per token we only need
       v_t   = z_t @ (gamma*Bm)  = x_t @ P1 + c1        (P1 = W^T diag(g) Bm)
       mu_t  = x_t @ m + bbar                           (m = W^T 1 / D)
       ssq_t = x_t (W^T W) x_t^T + 2 x_t (W^T b) + b.b  (row quadratic form)
       u_t   = s_t * v_t + (-mu_t s_t) * g + bbeta,  s_t = rsqrt(var+eps)
   so the only big matmul is x @ [W^T W | P1 | m | pad | 2 W^T b]
   ([768 x 865]), evaluated as q^T = wcat^T @ x^T with d-tile-major order
   so TensorE streams directly behind the per-tile DMAs.

3. The truncated scan h = sum_t u_t A^{T_EFF-1-t} uses two-level chunking
   t = L1*j + l:   h = sum_j ( sum_l u_{L1 j + l} A^{L1-1-l} ) (A^L1)^{L2-1-j}
   which is L1 + L2 small matmuls with the chunk index living in the free
   dim (no data rearrangement needed).

Sharding: data-parallel over batch, B=32 -> 4 per core x 8 cores.
Params (derived weights) replicated; no collectives; host gathers outputs.
"""

import numpy as np

import concourse.bacc as bacc
import concourse.mybir as mybir
import concourse.tile as tile
from concourse.bass_utils import run_bass_kernel_spmd

F32 = mybir.dt.float32
F32R = mybir.dt.float32r

B, T, D, N = 32, 2048, 768, 64
N_CORES = 8
B_LOC = B // N_CORES
L1 = 8
LN_EPS = 1e-5
NORM_EPS = 1e-12
TOKB = 256          # tokens per stage-1/2/3 block (keeps f32r fast path, Nf=256)
NCOLS = 865         # [ M(768) | P1(64) | m(1) | pad(31) | 2wb(1) ]
NCH = 7             # column chunks of <=128

LAST_RESULTS = None  # BassKernelResults of the most recent run (for test harness)
LAST_NC = None


def _choose_t_eff(A64):
    """Smallest T_EFF whose dropped tail is negligible: ||A^k|| * T < 1e-9."""
    for t_eff in (64, 128, 256, 512):
        nrm = np.linalg.norm(np.linalg.matrix_power(A64, t_eff), 2)
        if nrm * T < 1e-9:
            return t_eff
    return 512


def _build_bass(t_eff, weights):
    """Build the single-core Bass program (same NEFF runs SPMD on all cores)."""
    wcat, apow1, apow2, cmat, cols3, bbar, bias_eps = weights
    L2 = t_eff // L1
    TOK = B_LOC * t_eff
    NB = TOK // TOKB
    assert wcat.shape[1] == NCOLS and TOK % TOKB == 0

    nc = bacc.Bacc("TRN2", target_bir_lowering=False)

    # blob_f32:  [64, 2*L1*N + L2*N + 3] = apow1 | apow2 | cols3
    # blob_f32r: [128, 769] = cmat(rows 0:64) + ones1(row 64) | onescol(col 768)
    # dt{i}:     [128, NCOLS + TOK] = wcat rows | x^T rows   (per d-tile)
    BF = L1 * N + L2 * N + 3
    blobf_d = nc.dram_tensor("blob_f32", [N, BF], F32, kind="ExternalInput")
    blobr_d = nc.dram_tensor("blob_f32r", [128, D + 65], F32R,
                             kind="ExternalInput")
    dt_d = [nc.dram_tensor(f"dt{i}", [128, NCOLS + TOK], F32R,
                           kind="ExternalInput") for i in range(6)]
    out_d = nc.dram_tensor("out", [B_LOC, D], F32, kind="ExternalOutput")

    with tile.TileContext(nc) as tc:
        with (
            tc.tile_pool(name="const", bufs=1) as const,
            tc.tile_pool(name="work", bufs=2) as work,
            tc.tile_pool(name="small", bufs=4 * NB) as small,
            tc.tile_pool(name="ps", bufs=8, space="PSUM") as ps,
        ):
            # ---- constant loads: 8 blob DMAs split over SP and ACT DGEs ---
            blobf_sb = const.tile([N, BF], F32, tag="blobf")
            nc.sync.dma_start(out=blobf_sb, in_=blobf_d[:, :])
            blobr_sb = const.tile([128, D + 65], F32R, tag="blobr")
            nc.scalar.dma_start(out=blobr_sb, in_=blobr_d[:, :])
            apow1_sb = blobf_sb[:, 0:L1 * N]
            apow2_sb = blobf_sb[:, L1 * N:L1 * N + L2 * N]
            cols3_sb = blobf_sb[:, L1 * N + L2 * N:BF]
            cmat_sb = blobr_sb[0:N, 0:D]
            ones1_sb = blobr_sb[0:1, D + 1:D + 65]
            onescol_sb = blobr_sb[:, D:D + 1]

            dtb = []
            for dt in range(6):
                eng = nc.sync if dt % 2 == 0 else nc.scalar
                t = const.tile([128, NCOLS + TOK], F32R, tag=f"dtb{dt}")
                eng.dma_start(out=t, in_=dt_d[dt][:, :])
                dtb.append(t)
            wcat_dt = [t[:, 0:NCOLS] for t in dtb]
            xt_dt = [t[:, NCOLS:NCOLS + TOK] for t in dtb]

            epsb = const.tile([1, 1], F32, tag="epsb")
            nc.vector.memset(epsb, bias_eps)
            zero4 = const.tile([B_LOC, 1], F32, tag="zero4")
            nc.vector.memset(zero4, 0.0)

            wT_sb = const.tile([N, TOK], F32, tag="wT")

            # ---- stages 1-3, per token block ------------------------------
            for blk in range(NB):
                tsl = slice(blk * TOKB, (blk + 1) * TOKB)

                # stage 1: q^T = wcat^T @ x^T.  dt-major so each d-tile's
                # matmuls start as soon as that tile's DMA lands.
                q_ps = [ps.tile([128, TOKB], F32, tag="ps", name=f"qp{c}")
                        for c in range(NCH)]
                for dt in range(6):
                    for c in range(NCH):
                        w = min(128, NCOLS - c * 128)
                        nc.tensor.matmul(
                            out=q_ps[c][0:w, :],
                            lhsT=wcat_dt[dt][:, c * 128:c * 128 + w],
                            rhs=xt_dt[dt][:, tsl],
                            start=(dt == 0),
                            stop=(dt == 5),
                        )

                # stage 2: ssq = sum_d xT * q1T  (elementwise + ones-matmul)
                ssq_ps = ps.tile([1, TOKB], F32, tag="ps")
                prod_sb = work.tile([128, 6, TOKB], F32R, tag="prod")
                for dt in range(6):
                    nc.vector.tensor_mul(
                        out=prod_sb[:, dt, :],
                        in0=xt_dt[dt][:, tsl],
                        in1=q_ps[dt][:, :],
                    )
                for dt in range(6):
                    nc.tensor.matmul(
                        out=ssq_ps[:, :],
                        lhsT=onescol_sb[:, :],
                        rhs=prod_sb[:, dt, :],
                        start=(dt == 0),
                        stop=(dt == 5),
                    )

                # stage 3: per-token scalars on [1, TOKB] rows
                # q6 rows: 0..63 = v^T, 64 = x@m, 96 = 2 x@wb
                q6 = q_ps[6]
                mu = small.tile([1, TOKB], F32, tag="mu")
                nc.vector.tensor_scalar_add(
                    out=mu, in0=q6[64:65, :], scalar1=float(bbar))
                msq = small.tile([1, TOKB], F32, tag="msq")
                nc.vector.tensor_mul(out=msq, in0=mu, in1=mu)
                # var = ssq/D + (2 x@wb)/D - mu^2, one PSUM operand per op
                t1 = small.tile([1, TOKB], F32, tag="t1")
                nc.vector.scalar_tensor_tensor(
                    out=t1, in0=q6[96:97, :], scalar=1.0 / D, in1=msq,
                    op0=mybir.AluOpType.mult, op1=mybir.AluOpType.subtract,
                )
                var = small.tile([1, TOKB], F32, tag="var")
                nc.vector.scalar_tensor_tensor(
                    out=var, in0=ssq_ps[0:1, :], scalar=1.0 / D, in1=t1,
                    op0=mybir.AluOpType.mult, op1=mybir.AluOpType.add,
                )
                # s = 1/sqrt(var + (bb/D + eps));  a = -mu * s
                std = small.tile([1, TOKB], F32, tag="std")
                nc.scalar.activation(
                    out=std, in_=var, func=mybir.ActivationFunctionType.Sqrt,
                    bias=epsb[:, :], scale=1.0)
                srow = small.tile([1, TOKB], F32R, tag="srow")
                with nc.allow_low_precision(reason="f32r output is fp32 bits"):
                    nc.vector.reciprocal(out=srow, in_=std)
                arow = small.tile([1, TOKB], F32R, tag="arow")
                nc.vector.scalar_tensor_tensor(
                    out=arow, in0=mu, scalar=-1.0, in1=srow,
                    op0=mybir.AluOpType.mult, op1=mybir.AluOpType.mult,
                )

                # broadcast s,a across 64 partitions via K=1 matmuls
                s64_ps = ps.tile([N, TOKB], F32, tag="ps")
                nc.tensor.matmul(out=s64_ps, lhsT=ones1_sb, rhs=srow,
                                 start=True, stop=True)
                a64_ps = ps.tile([N, TOKB], F32, tag="ps")
                nc.tensor.matmul(out=a64_ps, lhsT=ones1_sb, rhs=arow,
                                 start=True, stop=True)

                # w^T = s*(v^T + c1) + a*g + bbeta   [64, TOKB]
                wtmp = work.tile([N, TOKB], F32, tag="wtmp")
                nc.vector.tensor_scalar_add(
                    out=wtmp, in0=q6[0:64, :], scalar1=cols3_sb[:, 0:1])
                nc.vector.tensor_mul(out=wtmp, in0=wtmp, in1=s64_ps)
                nc.vector.scalar_tensor_tensor(
                    out=wtmp, in0=a64_ps, scalar=cols3_sb[:, 1:2], in1=wtmp,
                    op0=mybir.AluOpType.mult, op1=mybir.AluOpType.add,
                )
                nc.vector.tensor_scalar_add(
                    out=wT_sb[:, tsl], in0=wtmp, scalar1=cols3_sb[:, 2:3])

            # ---- stage 4: truncated scan as two-level chunked matmuls -----
            # tok = b*t_eff + j*L1 + l
            wT_v = wT_sb[:, :].rearrange(
                "n (b j l) -> n b j l", b=B_LOC, j=L2, l=L1)
            s_ps = ps.tile([N, B_LOC, L2], F32, tag="ps")
            for l in range(L1):
                nc.tensor.matmul(
                    out=s_ps,
                    lhsT=apow1_sb[:, l * N:(l + 1) * N],
                    rhs=wT_v[:, :, :, l],
                    start=(l == 0), stop=(l == L1 - 1),
                )
            s_sb = small.tile([N, B_LOC, L2], F32, tag="s_sb")
            nc.vector.tensor_copy(out=s_sb, in_=s_ps)

            h_ps = ps.tile([N, B_LOC], F32, tag="ps")
            for j in range(L2):
                nc.tensor.matmul(
                    out=h_ps,
                    lhsT=apow2_sb[:, j * N:(j + 1) * N],
                    rhs=s_sb[:, :, j],
                    start=(j == 0), stop=(j == L2 - 1),
                )
            h_sb = small.tile([N, B_LOC], F32R, tag="h_sb")
            nc.vector.tensor_copy(out=h_sb, in_=h_ps)

            # ---- stage 5: y = h^T @ C (f32r), row-normalize ---------------
            y_sb = work.tile([B_LOC, D], F32, tag="y")
            for half in range(2):
                esl = slice(half * 384, (half + 1) * 384)
                y_ps = ps.tile([B_LOC, 384], F32, tag="ps")
                nc.tensor.matmul(out=y_ps, lhsT=h_sb, rhs=cmat_sb[:, esl],
                                 start=True, stop=True)
                nc.vector.tensor_copy(out=y_sb[:, esl], in_=y_ps)

            scr = work.tile([B_LOC, D], F32, tag="scr")
            ssum = small.tile([B_LOC, 1], F32, tag="ssum")
            nc.vector.scalar_tensor_tensor(
                out=scr, in0=y_sb, scalar=1.0, in1=y_sb,
                op0=mybir.AluOpType.mult, op1=mybir.AluOpType.mult,
                accum_out=ssum,
            )
            nrm = small.tile([B_LOC, 1], F32, tag="nrm")
            nc.scalar.activation(out=nrm, in_=ssum,
                                 func=mybir.ActivationFunctionType.Sqrt,
                                 bias=zero4[:, :])
            nc.vector.tensor_scalar_max(out=nrm, in0=nrm, scalar1=NORM_EPS)
            nc.vector.reciprocal(out=nrm, in_=nrm)
            nc.vector.tensor_scalar_mul(out=y_sb, in0=y_sb, scalar1=nrm)

            nc.sync.dma_start(out=out_d[:, :], in_=y_sb)

    if not nc.is_finalized():
        nc.finalize()
    return nc


def prepare(inputs):
    """Host-side derived weights (fp64 -> fp32) keyed for _build_bass."""
    f64 = np.float64
    W64 = np.asarray(inputs["W_lin"], f64)
    b64 = np.asarray(inputs["b_lin"], f64)
    g64 = np.asarray(inputs["gamma"], f64)
    be64 = np.asarray(inputs["beta"], f64)
    A64 = np.asarray(inputs["A"], f64)
    Bm64 = np.asarray(inputs["Bm"], f64)
    C32 = np.asarray(inputs["C"], np.float32)

    t_eff = _choose_t_eff(A64)
    L2 = t_eff // L1

    G = g64[:, None] * Bm64
    P1 = W64.T @ G                               # [D, N]
    c1 = b64 @ G                                 # [N]
    mcol = W64.sum(axis=0) / D                   # [D]
    bbar = float(b64.mean())
    M = W64.T @ W64                              # [D, D]
    wb = W64.T @ b64                             # [D]
    bb = float(b64 @ b64)
    gv = g64 @ Bm64                              # [N]
    bbeta = be64 @ Bm64                          # [N]
    wcat = np.ascontiguousarray(np.concatenate(
        [M, P1, mcol[:, None], np.zeros((D, 31)), (2.0 * wb)[:, None]],
        axis=1).astype(np.float32))              # [768, 865]
    cols3 = np.ascontiguousarray(
        np.stack([c1, gv, bbeta], axis=1).astype(np.float32))  # [N, 3]
    bias_eps = float(bb / D + LN_EPS)

    Apows = [np.eye(N)]
    for _ in range(L1):
        Apows.append(Apows[-1] @ A64)
    apow1 = np.ascontiguousarray(np.concatenate(
        [Apows[L1 - 1 - l] for l in range(L1)], axis=1).astype(np.float32))
    A_L1 = Apows[L1]
    apow2 = np.ascontiguousarray(np.concatenate(
        [np.linalg.matrix_power(A_L1, L2 - 1 - j) for j in range(L2)],
        axis=1).astype(np.float32))

    return {
        "t_eff": t_eff,
        "weights": (wcat, apow1, apow2, C32, cols3, bbar, bias_eps),
    }


def make_in_maps(x, prep):
    t_eff = prep["t_eff"]
    TOK = B_LOC * t_eff
    wcat, apow1, apow2, C32, cols3, bbar, bias_eps = prep["weights"]

    blobf = np.ascontiguousarray(
        np.concatenate([apow1, apow2, cols3], axis=1).astype(np.float32))
    blobr = np.zeros((128, D + 65), np.float32)
    blobr[0:N, 0:D] = C32
    blobr[:, D] = 1.0             # onescol
    blobr[0, D + 1:D + 65] = 1.0  # ones1 row
    blobr = np.ascontiguousarray(blobr)

    in_maps = []
    for core in range(N_CORES):
        xs = x[core * B_LOC:(core + 1) * B_LOC, T - t_eff:, :]
        xT = np.ascontiguousarray(xs.reshape(TOK, D).T)
        m = {"blob_f32": blobf, "blob_f32r": blobr}
        for dt in range(6):
            m[f"dt{dt}"] = np.ascontiguousarray(np.concatenate(
                [wcat[dt * 128:(dt + 1) * 128, :],
                 xT[dt * 128:(dt + 1) * 128, :]], axis=1).astype(np.float32))
        in_maps.append(m)
    return in_maps


def kernel(x, W_lin, b_lin, gamma, beta, A, Bm, C):
    global LAST_RESULTS, LAST_NC
    x = np.asarray(x, np.float32)
    assert x.shape == (B, T, D), x.shape

    prep = prepare(dict(W_lin=W_lin, b_lin=b_lin, gamma=gamma, beta=beta,
                        A=A, Bm=Bm, C=C))
    nc = _build_bass(prep["t_eff"], prep["weights"])
    in_maps = make_in_maps(x, prep)

    LAST_NC = nc
    res = run_bass_kernel_spmd(nc, in_maps, core_ids=list(range(N_CORES)))
    LAST_RESULTS = res
    out = np.concatenate([r["out"] for r in res.results], axis=0)
    return out.astype(np.float32)


# revision 25
# speedup vs baseline: 1.0731x; 1.0731x over previous
"""Trainium2 Bass kernel for nn_CustomS4.

Pipeline computed by the reference:
    z   = x @ W^T + b                      adapter Linear      [B,T,D]
    xh  = LN(z) * gamma + beta             LayerNorm over D
    u   = xh @ Bm                          input projection    [B,T,N]
    h_T = sum_t u_t A^{T-1-t}              linear scan, final state only
    out = normalize_rows(h_T @ C)          [B, D]

Key reformulations (all verified against the reference to ~1e-6 rel):

1. Only the FINAL scan state is needed and ||A^k|| decays like rho^k with
   rho = spectral_radius(A) ~ 0.5 (A = 0.5/sqrt(N) * randn), so the scan
   truncates to the last T_EFF timesteps with error below fp32 noise.
   T_EFF is chosen on the host from the actual decay of ||A^k||.

2. LayerNorm folds into the weights: per token we only need
       v_t   = z_t @ (gamma*Bm)  = x_t @ P1 + c1        (P1 = W^T diag(g) Bm)
       mu_t  = x_t @ m + bbar                           (m = W^T 1 / D)
       ssq_t = x_t (W^T W) x_t^T + 2 x_t (W^T b) + b.b  (row quadratic form)
       u_t   = s_t * v_t + (-mu_t s_t) * g + bbeta,  s_t = rsqrt(var+eps)
   so the only big matmul is x @ [W^T W | P1 | m | pad | 2 W^T b]
   ([768 x 865]), evaluated as q^T = wcat^T @ x^T with d-tile-major order
   so TensorE streams directly behind the per-tile DMAs.

3. The truncated scan h = sum_t u_t A^{T_EFF-1-t} uses two-level chunking
   t = L1*j + l:   h = sum_j ( sum_l u_{L1 j + l} A^{L1-1-l} ) (A^L1)^{L2-1-j}
   which is L1 + L2 small matmuls with the chunk index living in the free
   dim (no data rearrangement needed).

Sharding: data-parallel over batch, B=32 -> 4 per core x 8 cores.
Params (derived weights) replicated; no collectives; host gathers outputs.
"""

import numpy as np

import concourse.bacc as bacc
import concourse.mybir as mybir
import concourse.tile as tile
from concourse.bass_utils import run_bass_kernel_spmd

F32 = mybir.dt.float32
F32R = mybir.dt.float32r

B, T, D, N = 32, 2048, 768, 64
N_CORES = 8
B_LOC = B // N_CORES
L1 = 8
LN_EPS = 1e-5
NORM_EPS = 1e-12
TOKB = 256          # tokens per stage-1/2/3 block (keeps f32r fast path, Nf=256)
NCOLS = 865         # [ M(768) | P1(64) | m(1) | pad(31) | 2wb(1) ]
NCH = 7             # column chunks of <=128

LAST_RESULTS = None  # BassKernelResults of the most recent run (for test harness)
LAST_NC = None


def _choose_t_eff(A64):
    """Smallest T_EFF whose dropped tail is negligible: ||A^k|| * T < 1e-9."""
    for t_eff in (64, 128, 256, 512):
        nrm = np.linalg.norm(np.linalg.matrix_power(A64, t_eff), 2)
        if nrm * T < 1e-9:
            return t_eff
    return 512


def _build_bass(t_eff, weights):
    """Build the single-core Bass program (same NEFF runs SPMD on all cores)."""
    wcat, apow1, apow2, cmat, cols3, bbar, bias_eps = weights
    L2 = t_eff // L1
    TOK = B_LOC * t_eff
    NB = TOK // TOKB
    assert wcat.shape[1] == NCOLS and TOK % TOKB == 0

    nc = bacc.Bacc("TRN2", target_bir_lowering=False)

    # blob_f32:  [64, 2*L1*N + L2*N + 3] = apow1 | apow2 | cols3
    # blob_f32r: [128, 769] = cmat(rows 0:64) + ones1(row 64) | onescol(col 768)
    # dt{i}:     [128, NCOLS + TOK] = wcat rows | x^T rows   (per d-tile)
    BF = L1 * N + L2 * N + 3
    BFT = BF + N + 2   # + CC (C C^T) and two fp32 ones columns
    blobf_d = nc.dram_tensor("blob_f32", [N, BFT], F32, kind="ExternalInput")
    blobr_d = nc.dram_tensor("blob_f32r", [128, D + 65], F32R,
                             kind="ExternalInput")
    dt_d = [nc.dram_tensor(f"dt{i}", [128, NCOLS + TOK], F32R,
                           kind="ExternalInput") for i in range(6)]
    out_d = nc.dram_tensor("out", [B_LOC, D], F32, kind="ExternalOutput")

    with tile.TileContext(nc) as tc:
        with (
            tc.tile_pool(name="const", bufs=1) as const,
            tc.tile_pool(name="work", bufs=2) as work,
            tc.tile_pool(name="small", bufs=4 * NB) as small,
            tc.tile_pool(name="ps", bufs=8, space="PSUM") as ps,
        ):
            # ---- loads: 8 blob DMAs split over SP and ACT DGEs; the
            # stage-1-critical dt tiles go first, const blobs last ----
            dtb = []
            for dt in range(6):
                eng = nc.sync if dt % 2 == 0 else nc.scalar
                t = const.tile([128, NCOLS + TOK], F32R, tag=f"dtb{dt}")
                eng.dma_start(out=t, in_=dt_d[dt][:, :])
                dtb.append(t)
            wcat_dt = [t[:, 0:NCOLS] for t in dtb]
            xt_dt = [t[:, NCOLS:NCOLS + TOK] for t in dtb]

            blobf_sb = const.tile([N, BFT], F32, tag="blobf")
            nc.sync.dma_start(out=blobf_sb, in_=blobf_d[:, :])
            blobr_sb = const.tile([128, D + 65], F32R, tag="blobr")
            nc.scalar.dma_start(out=blobr_sb, in_=blobr_d[:, :])
            apow1_sb = blobf_sb[:, 0:L1 * N]
            apow2_sb = blobf_sb[:, L1 * N:L1 * N + L2 * N]
            cols3_sb = blobf_sb[:, L1 * N + L2 * N:BF]
            cc_sb = blobf_sb[:, BF:BF + N]
            ones32_sb = blobf_sb[:, BF + N:BF + N + 2]
            cmat_sb = blobr_sb[0:N, 0:D]
            ones1_sb = blobr_sb[0:1, D + 1:D + 65]
            onescol_sb = blobr_sb[:, D:D + 1]

            epsb = const.tile([1, 1], F32, tag="epsb")
            nc.vector.memset(epsb, bias_eps)
            bbarb = const.tile([1, 1], F32, tag="bbarb")
            nc.vector.memset(bbarb, bbar)
            zero4 = const.tile([B_LOC, 1], F32, tag="zero4")
            nc.vector.memset(zero4, 0.0)

            wT_sb = const.tile([N, TOK], F32, tag="wT")

            # ---- stages 1-3, per token block ------------------------------
            for blk in range(NB):
                tsl = slice(blk * TOKB, (blk + 1) * TOKB)

                # stage 1: q^T = wcat^T @ x^T.  dt-major so each d-tile's
                # matmuls start as soon as that tile's DMA lands.
                q_ps = [ps.tile([128, TOKB], F32, tag="ps", name=f"qp{c}")
                        for c in range(NCH)]
                for dt in range(6):
                    for c in range(NCH):
                        w = min(128, NCOLS - c * 128)
                        nc.tensor.matmul(
                            out=q_ps[c][0:w, :],
                            lhsT=wcat_dt[dt][:, c * 128:c * 128 + w],
                            rhs=xt_dt[dt][:, tsl],
                            start=(dt == 0),
                            stop=(dt == 5),
                        )

                # stage 2: ssq = sum_d xT * q1T  (elementwise + ones-matmul)
                ssq_ps = ps.tile([1, TOKB], F32, tag="ps")
                prod_sb = work.tile([128, 6, TOKB], F32R, tag="prod")
                for dt in range(6):
                    nc.vector.tensor_mul(
                        out=prod_sb[:, dt, :],
                        in0=xt_dt[dt][:, tsl],
                        in1=q_ps[dt][:, :],
                    )
                for dt in range(6):
                    nc.tensor.matmul(
                        out=ssq_ps[:, :],
                        lhsT=onescol_sb[:, :],
                        rhs=prod_sb[:, dt, :],
                        start=(dt == 0),
                        stop=(dt == 5),
                    )

                # stage 3: per-token scalars on [1, TOKB] rows
                # q6 rows: 0..63 = v^T, 64 = x@m, 96 = 2 x@wb
                q6 = q_ps[6]
                mu = small.tile([1, TOKB], F32, tag="mu")
                nc.vector.tensor_scalar_add(
                    out=mu, in0=q6[64:65, :], scalar1=float(bbar))
                msq = small.tile([1, TOKB], F32, tag="msq")
                nc.scalar.activation(
                    out=msq, in_=q6[64:65, :],
                    func=mybir.ActivationFunctionType.Square,
                    bias=bbarb[:, :], scale=1.0)
                # var = ssq/D + (2 x@wb)/D - mu^2, one PSUM operand per op
                t1 = small.tile([1, TOKB], F32, tag="t1")
                nc.vector.scalar_tensor_tensor(
                    out=t1, in0=q6[96:97, :], scalar=1.0 / D, in1=msq,
                    op0=mybir.AluOpType.mult, op1=mybir.AluOpType.subtract,
                )
                var = small.tile([1, TOKB], F32, tag="var")
                nc.vector.scalar_tensor_tensor(
                    out=var, in0=ssq_ps[0:1, :], scalar=1.0 / D, in1=t1,
                    op0=mybir.AluOpType.mult, op1=mybir.AluOpType.add,
                )
                # s = 1/sqrt(var + (bb/D + eps));  a = -mu * s
                std = small.tile([1, TOKB], F32, tag="std")
                nc.scalar.activation(
                    out=std, in_=var, func=mybir.ActivationFunctionType.Sqrt,
                    bias=epsb[:, :], scale=1.0)
                srow = small.tile([1, TOKB], F32R, tag="srow")
                with nc.allow_low_precision(reason="f32r output is fp32 bits"):
                    nc.vector.reciprocal(out=srow, in_=std)
                arow = small.tile([1, TOKB], F32R, tag="arow")
                nc.vector.scalar_tensor_tensor(
                    out=arow, in0=mu, scalar=-1.0, in1=srow,
                    op0=mybir.AluOpType.mult, op1=mybir.AluOpType.mult,
                )

                # broadcast s,a across 64 partitions via K=1 matmuls
                s64_ps = ps.tile([N, TOKB], F32, tag="ps")
                nc.tensor.matmul(out=s64_ps, lhsT=ones1_sb, rhs=srow,
                                 start=True, stop=True)
                a64_ps = ps.tile([N, TOKB], F32, tag="ps")
                nc.tensor.matmul(out=a64_ps, lhsT=ones1_sb, rhs=arow,
                                 start=True, stop=True)

                # w^T = s*(v^T + c1) + a*g + bbeta   [64, TOKB]
                wtmp = work.tile([N, TOKB], F32, tag="wtmp")
                nc.vector.tensor_scalar_add(
                    out=wtmp, in0=q6[0:64, :], scalar1=cols3_sb[:, 0:1])
                nc.vector.tensor_mul(out=wtmp, in0=wtmp, in1=s64_ps)
                nc.vector.scalar_tensor_tensor(
                    out=wtmp, in0=a64_ps, scalar=cols3_sb[:, 1:2], in1=wtmp,
                    op0=mybir.AluOpType.mult, op1=mybir.AluOpType.add,
                )
                nc.vector.tensor_scalar_add(
                    out=wT_sb[:, tsl], in0=wtmp, scalar1=cols3_sb[:, 2:3])

            # ---- stage 4: truncated scan as two-level chunked matmuls -----
            # tok = b*t_eff + j*L1 + l
            wT_v = wT_sb[:, :].rearrange(
                "n (b j l) -> n b j l", b=B_LOC, j=L2, l=L1)
            s_ps = ps.tile([N, B_LOC, L2], F32, tag="ps")
            for l in range(L1):
                nc.tensor.matmul(
                    out=s_ps,
                    lhsT=apow1_sb[:, l * N:(l + 1) * N],
                    rhs=wT_v[:, :, :, l],
                    start=(l == 0), stop=(l == L1 - 1),
                )
            s_sb = small.tile([N, B_LOC, L2], F32, tag="s_sb")
            nc.vector.tensor_copy(out=s_sb, in_=s_ps)

            h_ps = ps.tile([N, B_LOC], F32, tag="ps")
            for j in range(L2):
                nc.tensor.matmul(
                    out=h_ps,
                    lhsT=apow2_sb[:, j * N:(j + 1) * N],
                    rhs=s_sb[:, :, j],
                    start=(j == 0), stop=(j == L2 - 1),
                )
            h_sb = small.tile([N, B_LOC], F32R, tag="h_sb")
            nc.vector.tensor_copy(out=h_sb, in_=h_ps)
            h32_sb = small.tile([N, B_LOC], F32, tag="h32_sb")
            nc.vector.tensor_copy(out=h32_sb, in_=h_ps)

            # ||y_b||^2 = h_b (C C^T) h_b^T — computed while stage 5 runs
            hcc_ps = ps.tile([N, B_LOC], F32, tag="ps")
            nc.tensor.matmul(out=hcc_ps, lhsT=cc_sb, rhs=h32_sb,
                             start=True, stop=True)
            prod2 = small.tile([N, B_LOC], F32, tag="prod2")
            nc.vector.tensor_mul(out=prod2, in0=h32_sb, in1=hcc_ps)
            ssum_ps = ps.tile([B_LOC, 2], F32, tag="ps")
            nc.tensor.matmul(out=ssum_ps, lhsT=prod2, rhs=ones32_sb,
                             start=True, stop=True)
            nrm = small.tile([B_LOC, 1], F32, tag="nrm")
            nc.scalar.activation(out=nrm, in_=ssum_ps[:, 0:1],
                                 func=mybir.ActivationFunctionType.Sqrt,
                                 bias=zero4[:, :])
            nc.vector.tensor_scalar_max(out=nrm, in0=nrm, scalar1=NORM_EPS)
            rnrm = small.tile([B_LOC, 1], F32, tag="rnrm")
            nc.vector.reciprocal(out=rnrm, in_=nrm)

            # ---- stage 5: y = h^T @ C (f32r), scale by 1/||y||, DMA out --
            y_sb = work.tile([B_LOC, D], F32, tag="y")
            for half in range(2):
                esl = slice(half * 384, (half + 1) * 384)
                y_ps = ps.tile([B_LOC, 384], F32, tag="ps")
                nc.tensor.matmul(out=y_ps, lhsT=h_sb, rhs=cmat_sb[:, esl],
                                 start=True, stop=True)
                nc.vector.tensor_scalar_mul(
                    out=y_sb[:, esl], in0=y_ps, scalar1=rnrm)
                eng = nc.sync if half == 0 else nc.scalar
                eng.dma_start(out=out_d[:, esl], in_=y_sb[:, esl])

    if not nc.is_finalized():
        nc.finalize()
    return nc


def prepare(inputs):
    """Host-side derived weights (fp64 -> fp32) keyed for _build_bass."""
    f64 = np.float64
    W64 = np.asarray(inputs["W_lin"], f64)
    b64 = np.asarray(inputs["b_lin"], f64)
    g64 = np.asarray(inputs["gamma"], f64)
    be64 = np.asarray(inputs["beta"], f64)
    A64 = np.asarray(inputs["A"], f64)
    Bm64 = np.asarray(inputs["Bm"], f64)
    C32 = np.asarray(inputs["C"], np.float32)

    t_eff = _choose_t_eff(A64)
    L2 = t_eff // L1

    G = g64[:, None] * Bm64
    P1 = W64.T @ G                               # [D, N]
    c1 = b64 @ G                                 # [N]
    mcol = W64.sum(axis=0) / D                   # [D]
    bbar = float(b64.mean())
    M = W64.T @ W64                              # [D, D]
    wb = W64.T @ b64                             # [D]
    bb = float(b64 @ b64)
    gv = g64 @ Bm64                              # [N]
    bbeta = be64 @ Bm64                          # [N]
    wcat = np.ascontiguousarray(np.concatenate(
        [M, P1, mcol[:, None], np.zeros((D, 31)), (2.0 * wb)[:, None]],
        axis=1).astype(np.float32))              # [768, 865]
    cols3 = np.ascontiguousarray(
        np.stack([c1, gv, bbeta], axis=1).astype(np.float32))  # [N, 3]
    bias_eps = float(bb / D + LN_EPS)

    Apows = [np.eye(N)]
    for _ in range(L1):
        Apows.append(Apows[-1] @ A64)
    apow1 = np.ascontiguousarray(np.concatenate(
        [Apows[L1 - 1 - l] for l in range(L1)], axis=1).astype(np.float32))
    A_L1 = Apows[L1]
    apow2 = np.ascontiguousarray(np.concatenate(
        [np.linalg.matrix_power(A_L1, L2 - 1 - j) for j in range(L2)],
        axis=1).astype(np.float32))

    return {
        "t_eff": t_eff,
        "weights": (wcat, apow1, apow2, C32, cols3, bbar, bias_eps),
    }


def make_in_maps(x, prep):
    t_eff = prep["t_eff"]
    TOK = B_LOC * t_eff
    wcat, apow1, apow2, C32, cols3, bbar, bias_eps = prep["weights"]

    CC = (np.asarray(C32, np.float64) @ np.asarray(C32, np.float64).T)
    blobf = np.ascontiguousarray(np.concatenate(
        [apow1, apow2, cols3, CC.astype(np.float32), np.ones((N, 2))],
        axis=1).astype(np.float32))
    blobr = np.zeros((128, D + 65), np.float32)
    blobr[0:N, 0:D] = C32
    blobr[:, D] = 1.0             # onescol
    blobr[0, D + 1:D + 65] = 1.0  # ones1 row
    blobr = np.ascontiguousarray(blobr)

    in_maps = []
    for core in range(N_CORES):
        xs = x[core * B_LOC:(core + 1) * B_LOC, T - t_eff:, :]
        xT = np.ascontiguousarray(xs.reshape(TOK, D).T)
        m = {"blob_f32": blobf, "blob_f32r": blobr}
        for dt in range(6):
            m[f"dt{dt}"] = np.ascontiguousarray(np.concatenate(
                [wcat[dt * 128:(dt + 1) * 128, :],
                 xT[dt * 128:(dt + 1) * 128, :]], axis=1).astype(np.float32))
        in_maps.append(m)
    return in_maps


def kernel(x, W_lin, b_lin, gamma, beta, A, Bm, C):
    global LAST_RESULTS, LAST_NC
    x = np.asarray(x, np.float32)
    assert x.shape == (B, T, D), x.shape

    prep = prepare(dict(W_lin=W_lin, b_lin=b_lin, gamma=gamma, beta=beta,
                        A=A, Bm=Bm, C=C))
    nc = _build_bass(prep["t_eff"], prep["weights"])
    in_maps = make_in_maps(x, prep)

    LAST_NC = nc
    res = run_bass_kernel_spmd(nc, in_maps, core_ids=list(range(N_CORES)))
    LAST_RESULTS = res
    out = np.concatenate([r["out"] for r in res.results], axis=0)
    return out.astype(np.float32)


# revision 27
# speedup vs baseline: 1.0880x; 1.0139x over previous
"""Trainium2 Bass kernel for nn_CustomS4.

Pipeline computed by the reference:
    z   = x @ W^T + b                      adapter Linear      [B,T,D]
    xh  = LN(z) * gamma + beta             LayerNorm over D
    u   = xh @ Bm                          input projection    [B,T,N]
    h_T = sum_t u_t A^{T-1-t}              linear scan, final state only
    out = normalize_rows(h_T @ C)          [B, D]

Key reformulations (all verified against the reference to ~1e-6 rel):

1. Only the FINAL scan state is needed and ||A^k|| decays like rho^k with
   rho = spectral_radius(A) ~ 0.5 (A = 0.5/sqrt(N) * randn), so the scan
   truncates to the last T_EFF timesteps with error below fp32 noise.
   T_EFF is chosen on the host from the actual decay of ||A^k||.

2. LayerNorm folds into the weights: per token we only need
       v_t   = z_t @ (gamma*Bm)  = x_t @ P1 + c1        (P1 = W^T diag(g) Bm)
       mu_t  = x_t @ m + bbar                           (m = W^T 1 / D)
       ssq_t = x_t (W^T W) x_t^T + 2 x_t (W^T b) + b.b  (row quadratic form)
       u_t   = s_t * v_t + (-mu_t s_t) * g + bbeta,  s_t = rsqrt(var+eps)
   so the only big matmul is x @ [W^T W | P1 | m | pad | 2 W^T b]
   ([768 x 865]), evaluated as q^T = wcat^T @ x^T with d-tile-major order
   so TensorE streams directly behind the per-tile DMAs.

3. The truncated scan h = sum_t u_t A^{T_EFF-1-t} uses two-level chunking
   t = L1*j + l:   h = sum_j ( sum_l u_{L1 j + l} A^{L1-1-l} ) (A^L1)^{L2-1-j}
   which is L1 + L2 small matmuls with the chunk index living in the free
   dim (no data rearrangement needed).

Sharding: data-parallel over batch, B=32 -> 4 per core x 8 cores.
Params (derived weights) replicated; no collectives; host gathers outputs.
"""

import numpy as np

import concourse.bacc as bacc
import concourse.mybir as mybir
import concourse.tile as tile
from concourse.bass_utils import run_bass_kernel_spmd

F32 = mybir.dt.float32
F32R = mybir.dt.float32r

B, T, D, N = 32, 2048, 768, 64
N_CORES = 8
B_LOC = B // N_CORES
L1 = 8
LN_EPS = 1e-5
NORM_EPS = 1e-12
TOKB = 256          # tokens per stage-1/2/3 block (keeps f32r fast path, Nf=256)
NCOLS = 865         # [ M(768) | P1(64) | m(1) | pad(31) | 2wb(1) ]
NCH = 7             # column chunks of <=128

LAST_RESULTS = None  # BassKernelResults of the most recent run (for test harness)
LAST_NC = None


def _choose_t_eff(A64):
    """Smallest T_EFF whose dropped tail is negligible: ||A^k|| * T < 1e-9."""
    for t_eff in (64, 128, 256, 512):
        nrm = np.linalg.norm(np.linalg.matrix_power(A64, t_eff), 2)
        if nrm * T < 1e-9:
            return t_eff
    return 512


def _build_bass(t_eff, weights):
    """Build the single-core Bass program (same NEFF runs SPMD on all cores)."""
    wcat, apow1, apow2, cmat, cols4, bbar, bias_eps = weights
    L2 = t_eff // L1
    TOK = B_LOC * t_eff
    NB = TOK // TOKB
    assert wcat.shape[1] == NCOLS and TOK % TOKB == 0

    nc = bacc.Bacc("TRN2", target_bir_lowering=False)

    # blob_f32:  [64, 2*L1*N + L2*N + 3] = apow1 | apow2 | cols3
    # blob_f32r: [128, 769] = cmat(rows 0:64) + ones1(row 64) | onescol(col 768)
    # dt{i}:     [128, NCOLS + TOK] = wcat rows | x^T rows   (per d-tile)
    BF = L1 * N + L2 * N + 4
    BFT = BF + N + 2   # + CC (C C^T) and two fp32 ones columns
    blobf_d = nc.dram_tensor("blob_f32", [N, BFT], F32, kind="ExternalInput")
    blobr_d = nc.dram_tensor("blob_f32r", [128, D + 65], F32R,
                             kind="ExternalInput")
    dt_d = [nc.dram_tensor(f"dt{i}", [128, NCOLS + TOK], F32R,
                           kind="ExternalInput") for i in range(6)]
    out_d = nc.dram_tensor("out", [B_LOC, D], F32, kind="ExternalOutput")

    with tile.TileContext(nc) as tc:
        with (
            tc.tile_pool(name="const", bufs=1) as const,
            tc.tile_pool(name="work", bufs=2) as work,
            tc.tile_pool(name="small", bufs=4 * NB) as small,
            tc.tile_pool(name="ps", bufs=8, space="PSUM") as ps,
        ):
            # ---- loads: 8 blob DMAs split over SP and ACT DGEs; the
            # stage-1-critical dt tiles go first, const blobs last ----
            dtb = []
            for dt in range(6):
                eng = nc.sync if dt % 2 == 0 else nc.scalar
                t = const.tile([128, NCOLS + TOK], F32R, tag=f"dtb{dt}")
                eng.dma_start(out=t, in_=dt_d[dt][:, :])
                dtb.append(t)
            wcat_dt = [t[:, 0:NCOLS] for t in dtb]
            xt_dt = [t[:, NCOLS:NCOLS + TOK] for t in dtb]

            blobf_sb = const.tile([N, BFT], F32, tag="blobf")
            nc.sync.dma_start(out=blobf_sb, in_=blobf_d[:, :])
            blobr_sb = const.tile([128, D + 65], F32R, tag="blobr")
            nc.scalar.dma_start(out=blobr_sb, in_=blobr_d[:, :])
            apow1_sb = blobf_sb[:, 0:L1 * N]
            apow2_sb = blobf_sb[:, L1 * N:L1 * N + L2 * N]
            cols4_sb = blobf_sb[:, L1 * N + L2 * N:BF]
            cc_sb = blobf_sb[:, BF:BF + N]
            ones32_sb = blobf_sb[:, BF + N:BF + N + 2]
            cmat_sb = blobr_sb[0:N, 0:D]
            ones1_sb = blobr_sb[0:1, D + 1:D + 65]
            onescol_sb = blobr_sb[:, D:D + 1]

            epsb = const.tile([1, 1], F32, tag="epsb")
            nc.vector.memset(epsb, bias_eps)
            bbarb = const.tile([1, 1], F32, tag="bbarb")
            nc.vector.memset(bbarb, bbar)
            zero4 = const.tile([B_LOC, 1], F32, tag="zero4")
            nc.vector.memset(zero4, 0.0)

            wT_sb = const.tile([N, TOK], F32, tag="wT")

            # ---- stages 1-3, per token block ------------------------------
            for blk in range(NB):
                tsl = slice(blk * TOKB, (blk + 1) * TOKB)

                # stage 1: q^T = wcat^T @ x^T.  dt-major so each d-tile's
                # matmuls start as soon as that tile's DMA lands.
                q_ps = [ps.tile([128, TOKB], F32, tag="ps", name=f"qp{c}")
                        for c in range(NCH)]
                for dt in range(6):
                    for c in range(NCH):
                        w = min(128, NCOLS - c * 128)
                        nc.tensor.matmul(
                            out=q_ps[c][0:w, :],
                            lhsT=wcat_dt[dt][:, c * 128:c * 128 + w],
                            rhs=xt_dt[dt][:, tsl],
                            start=(dt == 0),
                            stop=(dt == 5),
                        )

                # stage 2: ssq = sum_d xT * q1T  (elementwise + ones-matmul)
                ssq_ps = ps.tile([1, TOKB], F32, tag="ps")
                prod_sb = work.tile([128, 6, TOKB], F32R, tag="prod")
                for dt in range(6):
                    nc.vector.tensor_mul(
                        out=prod_sb[:, dt, :],
                        in0=xt_dt[dt][:, tsl],
                        in1=q_ps[dt][:, :],
                    )
                for dt in range(6):
                    nc.tensor.matmul(
                        out=ssq_ps[:, :],
                        lhsT=onescol_sb[:, :],
                        rhs=prod_sb[:, dt, :],
                        start=(dt == 0),
                        stop=(dt == 5),
                    )

                # stage 3: per-token scalars on [1, TOKB] rows
                # q6 rows: 0..63 = v^T, 64 = x@m, 96 = 2 x@wb
                q6 = q_ps[6]
                mu = small.tile([1, TOKB], F32R, tag="mu")
                nc.vector.tensor_scalar_add(
                    out=mu, in0=q6[64:65, :], scalar1=float(bbar))
                msq = small.tile([1, TOKB], F32, tag="msq")
                nc.scalar.activation(
                    out=msq, in_=q6[64:65, :],
                    func=mybir.ActivationFunctionType.Square,
                    bias=bbarb[:, :], scale=1.0)
                # var = ssq/D + (2 x@wb)/D - mu^2, one PSUM operand per op
                t1 = small.tile([1, TOKB], F32, tag="t1")
                nc.vector.scalar_tensor_tensor(
                    out=t1, in0=q6[96:97, :], scalar=1.0 / D, in1=msq,
                    op0=mybir.AluOpType.mult, op1=mybir.AluOpType.subtract,
                )
                var = small.tile([1, TOKB], F32, tag="var")
                nc.vector.scalar_tensor_tensor(
                    out=var, in0=ssq_ps[0:1, :], scalar=1.0 / D, in1=t1,
                    op0=mybir.AluOpType.mult, op1=mybir.AluOpType.add,
                )
                # s = 1/sqrt(var + (bb/D + eps));  a = -mu * s
                std = small.tile([1, TOKB], F32, tag="std")
                nc.scalar.activation(
                    out=std, in_=var, func=mybir.ActivationFunctionType.Sqrt,
                    bias=epsb[:, :], scale=1.0)
                srow = small.tile([1, TOKB], F32R, tag="srow")
                with nc.allow_low_precision(reason="f32r output is fp32 bits"):
                    nc.vector.reciprocal(out=srow, in_=std)

                # broadcast s,mu across 64 partitions via K=1 matmuls
                s64_ps = ps.tile([N, TOKB], F32, tag="ps")
                nc.tensor.matmul(out=s64_ps, lhsT=ones1_sb, rhs=srow,
                                 start=True, stop=True)
                m64_ps = ps.tile([N, TOKB], F32, tag="ps")
                nc.tensor.matmul(out=m64_ps, lhsT=ones1_sb, rhs=mu,
                                 start=True, stop=True)

                # w^T = s * (v^T + c1 - g*mu); the constant bbeta term is
                # folded into hconst after the scan (it is w-independent).
                wtmp = work.tile([N, TOKB], F32, tag="wtmp")
                nc.vector.tensor_scalar_add(
                    out=wtmp, in0=q6[0:64, :], scalar1=cols4_sb[:, 0:1])
                nc.vector.scalar_tensor_tensor(
                    out=wtmp, in0=m64_ps, scalar=cols4_sb[:, 1:2], in1=wtmp,
                    op0=mybir.AluOpType.mult, op1=mybir.AluOpType.add,
                )
                nc.vector.tensor_mul(out=wT_sb[:, tsl], in0=wtmp, in1=s64_ps)

            # ---- stage 4: truncated scan as two-level chunked matmuls -----
            # tok = b*t_eff + j*L1 + l
            wT_v = wT_sb[:, :].rearrange(
                "n (b j l) -> n b j l", b=B_LOC, j=L2, l=L1)
            s_ps = ps.tile([N, B_LOC, L2], F32, tag="ps")
            for l in range(L1):
                nc.tensor.matmul(
                    out=s_ps,
                    lhsT=apow1_sb[:, l * N:(l + 1) * N],
                    rhs=wT_v[:, :, :, l],
                    start=(l == 0), stop=(l == L1 - 1),
                )
            s_sb = small.tile([N, B_LOC, L2], F32, tag="s_sb")
            nc.vector.tensor_copy(out=s_sb, in_=s_ps)

            h_ps = ps.tile([N, B_LOC], F32, tag="ps")
            for j in range(L2):
                nc.tensor.matmul(
                    out=h_ps,
                    lhsT=apow2_sb[:, j * N:(j + 1) * N],
                    rhs=s_sb[:, :, j],
                    start=(j == 0), stop=(j == L2 - 1),
                )
            h_sb = small.tile([N, B_LOC], F32R, tag="h_sb")
            nc.vector.tensor_scalar_add(
                out=h_sb, in0=h_ps, scalar1=cols4_sb[:, 2:3])
            h32_sb = small.tile([N, B_LOC], F32, tag="h32_sb")
            nc.vector.tensor_scalar_add(
                out=h32_sb, in0=h_ps, scalar1=cols4_sb[:, 2:3])

            # ||y_b||^2 = h_b (C C^T) h_b^T — computed while stage 5 runs
            hcc_ps = ps.tile([N, B_LOC], F32, tag="ps")
            nc.tensor.matmul(out=hcc_ps, lhsT=cc_sb, rhs=h32_sb,
                             start=True, stop=True)
            prod2 = small.tile([N, B_LOC], F32, tag="prod2")
            nc.vector.tensor_mul(out=prod2, in0=h32_sb, in1=hcc_ps)
            ssum_ps = ps.tile([B_LOC, 2], F32, tag="ps")
            nc.tensor.matmul(out=ssum_ps, lhsT=prod2, rhs=ones32_sb,
                             start=True, stop=True)
            nrm = small.tile([B_LOC, 1], F32, tag="nrm")
            nc.scalar.activation(out=nrm, in_=ssum_ps[:, 0:1],
                                 func=mybir.ActivationFunctionType.Sqrt,
                                 bias=zero4[:, :])
            nc.vector.tensor_scalar_max(out=nrm, in0=nrm, scalar1=NORM_EPS)
            rnrm = small.tile([B_LOC, 1], F32, tag="rnrm")
            nc.vector.reciprocal(out=rnrm, in_=nrm)

            # ---- stage 5: y = h^T @ C (f32r), scale by 1/||y||, DMA out --
            y_sb = work.tile([B_LOC, D], F32, tag="y")
            for half in range(2):
                esl = slice(half * 384, (half + 1) * 384)
                y_ps = ps.tile([B_LOC, 384], F32, tag="ps")
                nc.tensor.matmul(out=y_ps, lhsT=h_sb, rhs=cmat_sb[:, esl],
                                 start=True, stop=True)
                nc.vector.tensor_scalar_mul(
                    out=y_sb[:, esl], in0=y_ps, scalar1=rnrm)
                eng = nc.sync if half == 0 else nc.scalar
                eng.dma_start(out=out_d[:, esl], in_=y_sb[:, esl])

    if not nc.is_finalized():
        nc.finalize()
    return nc


def prepare(inputs):
    """Host-side derived weights (fp64 -> fp32) keyed for _build_bass."""
    f64 = np.float64
    W64 = np.asarray(inputs["W_lin"], f64)
    b64 = np.asarray(inputs["b_lin"], f64)
    g64 = np.asarray(inputs["gamma"], f64)
    be64 = np.asarray(inputs["beta"], f64)
    A64 = np.asarray(inputs["A"], f64)
    Bm64 = np.asarray(inputs["Bm"], f64)
    C32 = np.asarray(inputs["C"], np.float32)

    t_eff = _choose_t_eff(A64)
    L2 = t_eff // L1

    G = g64[:, None] * Bm64
    P1 = W64.T @ G                               # [D, N]
    c1 = b64 @ G                                 # [N]
    mcol = W64.sum(axis=0) / D                   # [D]
    bbar = float(b64.mean())
    M = W64.T @ W64                              # [D, D]
    wb = W64.T @ b64                             # [D]
    bb = float(b64 @ b64)
    gv = g64 @ Bm64                              # [N]
    bbeta = be64 @ Bm64                          # [N]
    wcat = np.ascontiguousarray(np.concatenate(
        [M, P1, mcol[:, None], np.zeros((D, 31)), (2.0 * wb)[:, None]],
        axis=1).astype(np.float32))              # [768, 865]
    Asum = np.zeros((N, N))
    Ak = np.eye(N)
    for _ in range(t_eff):
        Asum += Ak
        Ak = Ak @ A64
    hconst = bbeta @ Asum                        # [N]
    cols4 = np.ascontiguousarray(np.stack(
        [c1, -gv, hconst, np.zeros(N)], axis=1).astype(np.float32))  # [N, 4]
    bias_eps = float(bb / D + LN_EPS)

    Apows = [np.eye(N)]
    for _ in range(L1):
        Apows.append(Apows[-1] @ A64)
    apow1 = np.ascontiguousarray(np.concatenate(
        [Apows[L1 - 1 - l] for l in range(L1)], axis=1).astype(np.float32))
    A_L1 = Apows[L1]
    apow2 = np.ascontiguousarray(np.concatenate(
        [np.linalg.matrix_power(A_L1, L2 - 1 - j) for j in range(L2)],
        axis=1).astype(np.float32))

    return {
        "t_eff": t_eff,
        "weights": (wcat, apow1, apow2, C32, cols4, bbar, bias_eps),
    }


def make_in_maps(x, prep):
    t_eff = prep["t_eff"]
    TOK = B_LOC * t_eff
    wcat, apow1, apow2, C32, cols4, bbar, bias_eps = prep["weights"]

    CC = (np.asarray(C32, np.float64) @ np.asarray(C32, np.float64).T)
    blobf = np.ascontiguousarray(np.concatenate(
        [apow1, apow2, cols4, CC.astype(np.float32), np.ones((N, 2))],
        axis=1).astype(np.float32))
    blobr = np.zeros((128, D + 65), np.float32)
    blobr[0:N, 0:D] = C32
    blobr[:, D] = 1.0             # onescol
    blobr[0, D + 1:D + 65] = 1.0  # ones1 row
    blobr = np.ascontiguousarray(blobr)

    in_maps = []
    for core in range(N_CORES):
        xs = x[core * B_LOC:(core + 1) * B_LOC, T - t_eff:, :]
        xT = np.ascontiguousarray(xs.reshape(TOK, D).T)
        m = {"blob_f32": blobf, "blob_f32r": blobr}
        for dt in range(6):
            m[f"dt{dt}"] = np.ascontiguousarray(np.concatenate(
                [wcat[dt * 128:(dt + 1) * 128, :],
                 xT[dt * 128:(dt + 1) * 128, :]], axis=1).astype(np.float32))
        in_maps.append(m)
    return in_maps


def kernel(x, W_lin, b_lin, gamma, beta, A, Bm, C):
    global LAST_RESULTS, LAST_NC
    x = np.asarray(x, np.float32)
    assert x.shape == (B, T, D), x.shape

    prep = prepare(dict(W_lin=W_lin, b_lin=b_lin, gamma=gamma, beta=beta,
                        A=A, Bm=Bm, C=C))
    nc = _build_bass(prep["t_eff"], prep["weights"])
    in_maps = make_in_maps(x, prep)

    LAST_NC = nc
    res = run_bass_kernel_spmd(nc, in_maps, core_ids=list(range(N_CORES)))
    LAST_RESULTS = res
    out = np.concatenate([r["out"] for r in res.results], axis=0)
    return out.astype(np.float32)


# revision 29
# speedup vs baseline: 1.0962x; 1.0075x over previous
"""Trainium2 Bass kernel for nn_CustomS4.

Pipeline computed by the reference:
    z   = x @ W^T + b                      adapter Linear      [B,T,D]
    xh  = LN(z) * gamma + beta             LayerNorm over D
    u   = xh @ Bm                          input projection    [B,T,N]
    h_T = sum_t u_t A^{T-1-t}              linear scan, final state only
    out = normalize_rows(h_T @ C)          [B, D]

Key reformulations (all verified against the reference to ~1e-6 rel):

1. Only the FINAL scan state is needed and ||A^k|| decays like rho^k with
   rho = spectral_radius(A) ~ 0.5 (A = 0.5/sqrt(N) * randn), so the scan
   truncates to the last T_EFF timesteps with error below fp32 noise.
   T_EFF is chosen on the host from the actual decay of ||A^k||.

2. LayerNorm folds into the weights: per token we only need
       v_t   = z_t @ (gamma*Bm)  = x_t @ P1 + c1        (P1 = W^T diag(g) Bm)
       mu_t  = x_t @ m + bbar                           (m = W^T 1 / D)
       ssq_t = x_t (W^T W) x_t^T + 2 x_t (W^T b) + b.b  (row quadratic form)
       u_t   = s_t * v_t + (-mu_t s_t) * g + bbeta,  s_t = rsqrt(var+eps)
   so the only big matmul is x @ [W^T W | P1 | m | pad | 2 W^T b]
   ([768 x 865]), evaluated as q^T = wcat^T @ x^T with d-tile-major order
   so TensorE streams directly behind the per-tile DMAs.

3. The truncated scan h = sum_t u_t A^{T_EFF-1-t} uses two-level chunking
   t = L1*j + l:   h = sum_j ( sum_l u_{L1 j + l} A^{L1-1-l} ) (A^L1)^{L2-1-j}
   which is L1 + L2 small matmuls with the chunk index living in the free
   dim (no data rearrangement needed).

Sharding: data-parallel over batch, B=32 -> 4 per core x 8 cores.
Params (derived weights) replicated; no collectives; host gathers outputs.
"""

import numpy as np

import concourse.bacc as bacc
import concourse.mybir as mybir
import concourse.tile as tile
from concourse.bass_utils import run_bass_kernel_spmd

F32 = mybir.dt.float32
F32R = mybir.dt.float32r
BF16 = mybir.dt.bfloat16

B, T, D, N = 32, 2048, 768, 64
N_CORES = 8
B_LOC = B // N_CORES
L1 = 8
LN_EPS = 1e-5
NORM_EPS = 1e-12
TOKB = 256          # tokens per stage-1/2/3 block (keeps f32r fast path, Nf=256)
NCOLS = 865         # [ M(768) | P1(64) | m(1) | pad(31) | 2wb(1) ]
NCH = 7             # column chunks of <=128

LAST_RESULTS = None  # BassKernelResults of the most recent run (for test harness)
LAST_NC = None


def _choose_t_eff(A64):
    """Smallest T_EFF whose dropped tail is negligible: ||A^k|| * T < 1e-9."""
    for t_eff in (64, 128, 256, 512):
        nrm = np.linalg.norm(np.linalg.matrix_power(A64, t_eff), 2)
        if nrm * T < 1e-9:
            return t_eff
    return 512


def _build_bass(t_eff, weights):
    """Build the single-core Bass program (same NEFF runs SPMD on all cores)."""
    wcat, apow1, apow2, cmat, cols4, bbar, bias_eps = weights
    L2 = t_eff // L1
    TOK = B_LOC * t_eff
    NB = TOK // TOKB
    assert wcat.shape[1] == NCOLS and TOK % TOKB == 0

    nc = bacc.Bacc("TRN2", target_bir_lowering=False)

    # blob_f32:  [64, 2*L1*N + L2*N + 3] = apow1 | apow2 | cols3
    # blob_f32r: [128, 769] = cmat(rows 0:64) + ones1(row 64) | onescol(col 768)
    # dt{i}:     [128, NCOLS + TOK] = wcat rows | x^T rows   (per d-tile)
    BF = L1 * N + L2 * N + 4
    BFT = BF + N + 2   # + CC (C C^T) and two fp32 ones columns
    blobf_d = nc.dram_tensor("blob_f32", [N, BFT], F32, kind="ExternalInput")
    blobr_d = nc.dram_tensor("blob_f32r", [128, D + 65], F32R,
                             kind="ExternalInput")
    # Gram (M) block + its x copy in bf16 (feeds only the variance);
    # P1/m/wb block + its x copy in f32r (feeds v, mu directly).
    xwbf_d = [nc.dram_tensor(f"xwbf{i}", [128, 3, 768 + TOK], BF16,
                             kind="ExternalInput") for i in range(2)]
    xwfr_d = [nc.dram_tensor(f"xwfr{i}", [128, 3, 97 + TOK], F32R,
                             kind="ExternalInput") for i in range(2)]
    out_d = nc.dram_tensor("out", [B_LOC, D], F32, kind="ExternalOutput")

    with tile.TileContext(nc) as tc:
        with (
            tc.tile_pool(name="const", bufs=1) as const,
            tc.tile_pool(name="work", bufs=2) as work,
            tc.tile_pool(name="small", bufs=4 * NB) as small,
            tc.tile_pool(name="ps", bufs=8, space="PSUM") as ps,
        ):
            # ---- loads: 6 blob DMAs split over SP and ACT DGEs; the
            # stage-1-critical x/w blobs go first, const blobs last ----
            xwbf_sb = []
            xwfr_sb = []
            for i in range(2):
                eng = nc.sync if i == 0 else nc.scalar
                t = const.tile([128, 3, 768 + TOK], BF16, tag=f"xwbf{i}")
                eng.dma_start(out=t, in_=xwbf_d[i][:, :, :])
                xwbf_sb.append(t)
            for i in range(2):
                eng = nc.sync if i == 0 else nc.scalar
                t = const.tile([128, 3, 97 + TOK], F32R, tag=f"xwfr{i}")
                eng.dma_start(out=t, in_=xwfr_d[i][:, :, :])
                xwfr_sb.append(t)

            blobf_sb = const.tile([N, BFT], F32, tag="blobf")
            nc.sync.dma_start(out=blobf_sb, in_=blobf_d[:, :])
            blobr_sb = const.tile([128, D + 65], F32R, tag="blobr")
            nc.scalar.dma_start(out=blobr_sb, in_=blobr_d[:, :])

            def bfv(dt):   # bf16 view of d-tile dt: [M block | x^T]
                return xwbf_sb[dt % 2][:, dt // 2, :]

            def frv(dt):   # f32r view of d-tile dt: [P1|m|pad|wb | x^T]
                return xwfr_sb[dt % 2][:, dt // 2, :]
            apow1_sb = blobf_sb[:, 0:L1 * N]
            apow2_sb = blobf_sb[:, L1 * N:L1 * N + L2 * N]
            cols4_sb = blobf_sb[:, L1 * N + L2 * N:BF]
            cc_sb = blobf_sb[:, BF:BF + N]
            ones32_sb = blobf_sb[:, BF + N:BF + N + 2]
            cmat_sb = blobr_sb[0:N, 0:D]
            ones1_sb = blobr_sb[0:1, D + 1:D + 65]
            onescol_sb = blobr_sb[:, D:D + 1]

            epsb = const.tile([1, 1], F32, tag="epsb")
            nc.vector.memset(epsb, bias_eps)
            zero4 = const.tile([B_LOC, 1], F32, tag="zero4")
            nc.vector.memset(zero4, 0.0)

            wT_sb = const.tile([N, TOK], F32, tag="wT")

            # ---- stages 1-3, per token block ------------------------------
            for blk in range(NB):
                tsl = slice(blk * TOKB, (blk + 1) * TOKB)

                # stage 1: q^T = wcat^T @ x^T.  dt-major so each d-tile's
                # matmuls start as soon as that tile's DMA lands.
                # Chunks 0..5 (Gram -> variance only) run in bf16; chunk 6
                # (P1/m/wb -> v, mu) runs in f32r.
                q_ps = [ps.tile([128, TOKB], F32, tag="ps", name=f"qp{c}")
                        for c in range(NCH)]
                for dt in range(6):
                    bt = bfv(dt)
                    ft = frv(dt)
                    for c in range(6):
                        nc.tensor.matmul(
                            out=q_ps[c][:, :],
                            lhsT=bt[:, c * 128:(c + 1) * 128],
                            rhs=bt[:, 768 + blk * TOKB:768 + (blk + 1) * TOKB],
                            start=(dt == 0),
                            stop=(dt == 5),
                        )
                    nc.tensor.matmul(
                        out=q_ps[6][0:97, :],
                        lhsT=ft[:, 0:97],
                        rhs=ft[:, 97 + blk * TOKB:97 + (blk + 1) * TOKB],
                        start=(dt == 0),
                        stop=(dt == 5),
                    )

                # stage 2: ssq = sum_d xT * q1T  (elementwise + ones-matmul)
                ssq_ps = ps.tile([1, TOKB], F32, tag="ps")
                prod_sb = work.tile([128, 6, TOKB], F32R, tag="prod")
                for dt in range(6):
                    nc.vector.tensor_mul(
                        out=prod_sb[:, dt, :],
                        in0=bfv(dt)[:, 768 + blk * TOKB:768 + (blk + 1) * TOKB],
                        in1=q_ps[dt][:, :],
                    )
                for dt in range(6):
                    nc.tensor.matmul(
                        out=ssq_ps[:, :],
                        lhsT=onescol_sb[:, :],
                        rhs=prod_sb[:, dt, :],
                        start=(dt == 0),
                        stop=(dt == 5),
                    )

                # stage 3: per-token scalars on [1, TOKB] rows
                # q6 rows: 0..63 = v^T, 64 = x@m, 96 = 2 x@wb
                q6 = q_ps[6]
                mu = small.tile([1, TOKB], F32R, tag="mu")
                nc.vector.tensor_scalar_add(
                    out=mu, in0=q6[64:65, :], scalar1=float(bbar))
                msq = small.tile([1, TOKB], F32, tag="msq")
                nc.vector.tensor_mul(out=msq, in0=mu, in1=mu)
                # var = ssq/D + (2 x@wb)/D - mu^2, one PSUM operand per op
                t1 = small.tile([1, TOKB], F32, tag="t1")
                nc.vector.scalar_tensor_tensor(
                    out=t1, in0=q6[96:97, :], scalar=1.0 / D, in1=msq,
                    op0=mybir.AluOpType.mult, op1=mybir.AluOpType.subtract,
                )
                var = small.tile([1, TOKB], F32, tag="var")
                nc.vector.scalar_tensor_tensor(
                    out=var, in0=ssq_ps[0:1, :], scalar=1.0 / D, in1=t1,
                    op0=mybir.AluOpType.mult, op1=mybir.AluOpType.add,
                )
                # s = 1/sqrt(var + (bb/D + eps));  a = -mu * s
                std = small.tile([1, TOKB], F32, tag="std")
                nc.scalar.activation(
                    out=std, in_=var, func=mybir.ActivationFunctionType.Sqrt,
                    bias=epsb[:, :], scale=1.0)
                srow = small.tile([1, TOKB], F32R, tag="srow")
                with nc.allow_low_precision(reason="f32r output is fp32 bits"):
                    nc.vector.reciprocal(out=srow, in_=std)

                # broadcast s,mu across 64 partitions via K=1 matmuls
                s64_ps = ps.tile([N, TOKB], F32, tag="ps")
                nc.tensor.matmul(out=s64_ps, lhsT=ones1_sb, rhs=srow,
                                 start=True, stop=True)
                m64_ps = ps.tile([N, TOKB], F32, tag="ps")
                nc.tensor.matmul(out=m64_ps, lhsT=ones1_sb, rhs=mu,
                                 start=True, stop=True)

                # w^T = s * (v^T + c1 - g*mu); the constant bbeta term is
                # folded into hconst after the scan (it is w-independent).
                wtmp = work.tile([N, TOKB], F32, tag="wtmp")
                nc.vector.tensor_scalar_add(
                    out=wtmp, in0=q6[0:64, :], scalar1=cols4_sb[:, 0:1])
                nc.vector.scalar_tensor_tensor(
                    out=wtmp, in0=m64_ps, scalar=cols4_sb[:, 1:2], in1=wtmp,
                    op0=mybir.AluOpType.mult, op1=mybir.AluOpType.add,
                )
                nc.vector.tensor_mul(out=wT_sb[:, tsl], in0=wtmp, in1=s64_ps)

            # ---- stage 4: truncated scan as two-level chunked matmuls -----
            # tok = b*t_eff + j*L1 + l
            wT_v = wT_sb[:, :].rearrange(
                "n (b j l) -> n b j l", b=B_LOC, j=L2, l=L1)
            s_ps = ps.tile([N, B_LOC, L2], F32, tag="ps")
            for l in range(L1):
                nc.tensor.matmul(
                    out=s_ps,
                    lhsT=apow1_sb[:, l * N:(l + 1) * N],
                    rhs=wT_v[:, :, :, l],
                    start=(l == 0), stop=(l == L1 - 1),
                )
            s_sb = small.tile([N, B_LOC, L2], F32, tag="s_sb")
            nc.vector.tensor_copy(out=s_sb, in_=s_ps)

            h_ps = ps.tile([N, B_LOC], F32, tag="ps")
            for j in range(L2):
                nc.tensor.matmul(
                    out=h_ps,
                    lhsT=apow2_sb[:, j * N:(j + 1) * N],
                    rhs=s_sb[:, :, j],
                    start=(j == 0), stop=(j == L2 - 1),
                )
            h_sb = small.tile([N, B_LOC], F32R, tag="h_sb")
            nc.vector.tensor_scalar_add(
                out=h_sb, in0=h_ps, scalar1=cols4_sb[:, 2:3])
            h32_sb = small.tile([N, B_LOC], F32, tag="h32_sb")
            nc.vector.tensor_scalar_add(
                out=h32_sb, in0=h_ps, scalar1=cols4_sb[:, 2:3])

            # ||y_b||^2 = h_b (C C^T) h_b^T — computed while stage 5 runs
            hcc_ps = ps.tile([N, B_LOC], F32, tag="ps")
            nc.tensor.matmul(out=hcc_ps, lhsT=cc_sb, rhs=h32_sb,
                             start=True, stop=True)
            prod2 = small.tile([N, B_LOC], F32, tag="prod2")
            nc.vector.tensor_mul(out=prod2, in0=h32_sb, in1=hcc_ps)
            ssum_ps = ps.tile([B_LOC, 2], F32, tag="ps")
            nc.tensor.matmul(out=ssum_ps, lhsT=prod2, rhs=ones32_sb,
                             start=True, stop=True)
            nrm = small.tile([B_LOC, 1], F32, tag="nrm")
            nc.scalar.activation(out=nrm, in_=ssum_ps[:, 0:1],
                                 func=mybir.ActivationFunctionType.Sqrt,
                                 bias=zero4[:, :])
            nc.vector.tensor_scalar_max(out=nrm, in0=nrm, scalar1=NORM_EPS)
            rnrm = small.tile([B_LOC, 1], F32, tag="rnrm")
            nc.vector.reciprocal(out=rnrm, in_=nrm)

            # ---- stage 5: y = h^T @ C (f32r), scale by 1/||y||, DMA out --
            y_sb = work.tile([B_LOC, D], F32, tag="y")
            for half in range(2):
                esl = slice(half * 384, (half + 1) * 384)
                y_ps = ps.tile([B_LOC, 384], F32, tag="ps")
                nc.tensor.matmul(out=y_ps, lhsT=h_sb, rhs=cmat_sb[:, esl],
                                 start=True, stop=True)
                if half == 0:
                    nc.vector.tensor_scalar_mul(
                        out=y_sb[:, esl], in0=y_ps, scalar1=rnrm)
                else:
                    nc.scalar.activation(
                        out=y_sb[:, esl], in_=y_ps,
                        func=mybir.ActivationFunctionType.Copy,
                        bias=0.0, scale=rnrm)
                eng = nc.sync if half == 0 else nc.scalar
                eng.dma_start(out=out_d[:, esl], in_=y_sb[:, esl])

    if not nc.is_finalized():
        nc.finalize()
    return nc


def prepare(inputs):
    """Host-side derived weights (fp64 -> fp32) keyed for _build_bass."""
    f64 = np.float64
    W64 = np.asarray(inputs["W_lin"], f64)
    b64 = np.asarray(inputs["b_lin"], f64)
    g64 = np.asarray(inputs["gamma"], f64)
    be64 = np.asarray(inputs["beta"], f64)
    A64 = np.asarray(inputs["A"], f64)
    Bm64 = np.asarray(inputs["Bm"], f64)
    C32 = np.asarray(inputs["C"], np.float32)

    t_eff = _choose_t_eff(A64)
    L2 = t_eff // L1

    G = g64[:, None] * Bm64
    P1 = W64.T @ G                               # [D, N]
    c1 = b64 @ G                                 # [N]
    mcol = W64.sum(axis=0) / D                   # [D]
    bbar = float(b64.mean())
    M = W64.T @ W64                              # [D, D]
    wb = W64.T @ b64                             # [D]
    bb = float(b64 @ b64)
    gv = g64 @ Bm64                              # [N]
    bbeta = be64 @ Bm64                          # [N]
    wcat = np.ascontiguousarray(np.concatenate(
        [M, P1, mcol[:, None], np.zeros((D, 31)), (2.0 * wb)[:, None]],
        axis=1).astype(np.float32))              # [768, 865]
    Asum = np.zeros((N, N))
    Ak = np.eye(N)
    for _ in range(t_eff):
        Asum += Ak
        Ak = Ak @ A64
    hconst = bbeta @ Asum                        # [N]
    cols4 = np.ascontiguousarray(np.stack(
        [c1, -gv, hconst, np.zeros(N)], axis=1).astype(np.float32))  # [N, 4]
    bias_eps = float(bb / D + LN_EPS)

    Apows = [np.eye(N)]
    for _ in range(L1):
        Apows.append(Apows[-1] @ A64)
    apow1 = np.ascontiguousarray(np.concatenate(
        [Apows[L1 - 1 - l] for l in range(L1)], axis=1).astype(np.float32))
    A_L1 = Apows[L1]
    apow2 = np.ascontiguousarray(np.concatenate(
        [np.linalg.matrix_power(A_L1, L2 - 1 - j) for j in range(L2)],
        axis=1).astype(np.float32))

    return {
        "t_eff": t_eff,
        "weights": (wcat, apow1, apow2, C32, cols4, bbar, bias_eps),
    }


def make_in_maps(x, prep):
    t_eff = prep["t_eff"]
    TOK = B_LOC * t_eff
    wcat, apow1, apow2, C32, cols4, bbar, bias_eps = prep["weights"]

    CC = (np.asarray(C32, np.float64) @ np.asarray(C32, np.float64).T)
    blobf = np.ascontiguousarray(np.concatenate(
        [apow1, apow2, cols4, CC.astype(np.float32), np.ones((N, 2))],
        axis=1).astype(np.float32))
    blobr = np.zeros((128, D + 65), np.float32)
    blobr[0:N, 0:D] = C32
    blobr[:, D] = 1.0             # onescol
    blobr[0, D + 1:D + 65] = 1.0  # ones1 row
    blobr = np.ascontiguousarray(blobr)

    import ml_dtypes
    Mpart = wcat[:, 0:768]
    rest = wcat[:, 768:NCOLS]    # [768, 97] = P1|m|pad|2wb
    in_maps = []
    for core in range(N_CORES):
        xs = x[core * B_LOC:(core + 1) * B_LOC, T - t_eff:, :]
        xT = np.ascontiguousarray(xs.reshape(TOK, D).T)
        m = {"blob_f32": blobf, "blob_f32r": blobr}
        xwbf = np.empty((128, 6, 768 + TOK), ml_dtypes.bfloat16)
        xwfr = np.empty((128, 6, 97 + TOK), np.float32)
        for dt in range(6):
            rows = slice(dt * 128, (dt + 1) * 128)
            xwbf[:, dt, 0:768] = Mpart[rows, :].astype(ml_dtypes.bfloat16)
            xwbf[:, dt, 768:] = xT[rows, :].astype(ml_dtypes.bfloat16)
            xwfr[:, dt, 0:97] = rest[rows, :]
            xwfr[:, dt, 97:] = xT[rows, :]
        for i in range(2):
            m[f"xwbf{i}"] = np.ascontiguousarray(xwbf[:, i::2, :])
            m[f"xwfr{i}"] = np.ascontiguousarray(xwfr[:, i::2, :])
        in_maps.append(m)
    return in_maps


def kernel(x, W_lin, b_lin, gamma, beta, A, Bm, C):
    global LAST_RESULTS, LAST_NC
    x = np.asarray(x, np.float32)
    assert x.shape == (B, T, D), x.shape

    prep = prepare(dict(W_lin=W_lin, b_lin=b_lin, gamma=gamma, beta=beta,
                        A=A, Bm=Bm, C=C))
    nc = _build_bass(prep["t_eff"], prep["weights"])
    in_maps = make_in_maps(x, prep)

    LAST_NC = nc
    res = run_bass_kernel_spmd(nc, in_maps, core_ids=list(range(N_CORES)))
    LAST_RESULTS = res
    out = np.concatenate([r["out"] for r in res.results], axis=0)
    return out.astype(np.float32)
